# revision 2
# baseline (speedup 1.0000x reference)
"""DTS-SNN 2D Trainium2 kernel v2 (8 NeuronCores, batch-data-parallel).

Reference math:
  e in {0,1}; tr1/tr2 leaky traces; enc = blockconv4x4(unfold3x3((tr1-tr2)*SCALE))
  m_in LIF -> s_in; c_h = c_h*ds + s_in@Wh.T; m_h LIF -> s_h;
  c_o = c_o*ds + s_h@Wo.T; m_o LIF -> s_o; out = sum_t(s_o)/T

v2 changes over baseline:
  - temporal filter enc[t] = sum_tau (d1^(t-tau)-d2^(t-tau)) EC[tau] computed as a
    CASCADE of two first-order scans: z = scan_d2(scan_d1(EC*(d1-d2))); enc[t]=z[t-1].
    Removes one full DVE pass (the explicit a1+a2n add) and one buffer.
  - scan decay patterns DMAed from host instead of 70+ ScalarE copies.
  - t=0 timestep skipped everywhere (enc[0]=0 -> all spikes at t=0 are 0; sign
    cols memset to -1; chains run t=2..59 with m_1 read directly from input).
  - hidden weights prefetched into a deep SBUF ring during the conv phase.
  - output projection with stationary=W_out -> psum is [OUT, BT] directly
    (removes final transpose).
  - named scopes for per-phase HW timing.

Sharding: batch 16 -> 2 per core; all weights replicated (hint-compliant).
"""

import numpy as np

import concourse.bacc as bacc
import concourse.mybir as mybir
import concourse.tile as tile
from concourse.bass_utils import run_bass_kernel_spmd

# ---- model constants -------------------------------------------------------
B, T, H, W = 16, 60, 128, 128
NCORES = 8
BL = B // NCORES
HID, OUT = 512, 11
NCH = 96                    # feature chunks, one per (dj, j); 96 feats each
TH = 0.3
SCALE = 0.5
d1 = float(np.exp(-1.0 / 20.0))
d2 = float(np.exp(-1.0 / 5.0))
dm = float(np.exp(-1.0 / 20.0))
ds = float(np.exp(-1.0 / 5.0))
CONV_SC = 2.0 ** 6          # conv stationaries pre-scaled; evac applies 2^-6
LO_SC = 2.0 ** 11           # lo-residual scale for the hidden/output weights
FW = 136                    # frame: 4 residue planes x 34 (margins baked on host)
NFR = BL * T * 2            # 240 image planes per core
FGRP = 15                   # tau frames per conv psum group
NFG = T // FGRP
FB = 6                      # feature blocks = (dj, jhalf)
CHB = NCH // FB             # 16 chunks per feature block
BT = BL * T
WBUFS = 16                  # hidden-weight prefetch ring depth
TP = 80                     # padded timeline for the fused m_h/m_o chain
LAG = 18                    # m_o runs LAG steps behind m_h in the fused chain
f16 = mybir.dt.float16
f8 = mybir.dt.float8e4
f32 = mybir.dt.float32
A_ = mybir.AluOpType
F_ = mybir.ActivationFunctionType

_CACHE: dict = {}


def _build_program(debug_taps=False):
    nc = bacc.Bacc("TRN2", target_bir_lowering=False, debug=True)

    ev_d = nc.dram_tensor("ev", [BL * NFG * 2, 128, FGRP * FW], f8, kind="ExternalInput")
    ahi_d = nc.dram_tensor("ahi", [8, 128, 96], f16, kind="ExternalInput")
    alo_d = nc.dram_tensor("alo", [8, 128, 96], f16, kind="ExternalInput")
    whl_d = nc.dram_tensor("whl", [NCH // 2, 96, 4 * HID], f16, kind="ExternalInput")
    csh_d = nc.dram_tensor("csh", [4, 128], f32, kind="ExternalInput")
    wohi_d = nc.dram_tensor("wohi", [4, 128, OUT], f16, kind="ExternalInput")
    wolo_d = nc.dram_tensor("wolo", [4, 128, OUT], f16, kind="ExternalInput")
    cso_d = nc.dram_tensor("cso", [OUT], f32, kind="ExternalInput")
    id_d = nc.dram_tensor("ident", [128, 128], f32, kind="ExternalInput")
    pat_d = nc.dram_tensor("pats", [128, 2 * CHB * BL * T + 2 * TP], f32, kind="ExternalInput")
    out_d = nc.dram_tensor("out", [BL, OUT], f32, kind="ExternalOutput")
    taps = {}
    if debug_taps:
        for nm, shp in [("enc", [96, NCH * BT]), ("pj", [BT, HID]),
                        ("ch", [128, 4 * BT]), ("po", [OUT, BT])]:
            taps[nm] = nc.dram_tensor("tap_" + nm, shp, f32, kind="ExternalOutput")
        taps["sg"] = nc.dram_tensor("tap_sg", [96, NCH * BT], f16, kind="ExternalOutput")

    BLKF = CHB * BL * T
    with tile.TileContext(nc) as tc:
        with (
            tc.tile_pool(name="ev", bufs=1) as evp,
            tc.tile_pool(name="const", bufs=1) as cst,
            tc.tile_pool(name="acc", bufs=1) as accp,
            tc.tile_pool(name="state", bufs=1) as stp,
            tc.tile_pool(name="w", bufs=WBUFS) as wp,
            tc.tile_pool(name="cpsum", bufs=2, space="PSUM") as cps,
            tc.tile_pool(name="mpsum", bufs=1, space="PSUM") as mps,
            tc.tile_pool(name="tpsum", bufs=2, space="PSUM") as tps,
        ):
            # ---------------- constants / weights in SBUF ----------------
            ahi = cst.tile([128, 8 * 96], f16)
            alo = cst.tile([128, 8 * 96], f16)
            nc.sync.dma_start(ahi[:].rearrange("p (k m) -> p k m", k=8),
                              ahi_d[:].rearrange("k p m -> p k m"))
            nc.sync.dma_start(alo[:].rearrange("p (k m) -> p k m", k=8),
                              alo_d[:].rearrange("k p m -> p k m"))
            csh = cst.tile([128, 4], f32)
            nc.sync.dma_start(csh[:], csh_d[:].rearrange("k p -> p k"))
            wohi = cst.tile([128, 4 * OUT], f16)
            wolo = cst.tile([128, 4 * OUT], f16)
            nc.sync.dma_start(wohi[:].rearrange("p (k m) -> p k m", k=4),
                              wohi_d[:].rearrange("k p m -> p k m"))
            nc.sync.dma_start(wolo[:].rearrange("p (k m) -> p k m", k=4),
                              wolo_d[:].rearrange("k p m -> p k m"))
            cso = cst.tile([OUT, 1], f32)
            nc.sync.dma_start(cso[:], cso_d[:].rearrange("(p o) -> p o", o=1))
            negTH = cst.tile([128, 1], f32)
            nc.vector.memset(negTH[:], -TH)

            # ---------------- events -> SBUF (h on partitions) -----------
            # one tile per (b, t-group, channel): conv releases each tile
            # after its dj=2 pass, freeing the slot for hidden-weight tiles
            # that share the pool tag.
            evt = []
            for b in range(BL):
                for g in range(NFG):
                    for c in range(2):
                        t = evp.tile([128, 2048], f8, tag="evs",
                                     bufs=16, name=f"ev{b}{g}{c}")
                        nc.sync.dma_start(t[:, 0 : FGRP * FW],
                                          ev_d[(b * NFG + g) * 2 + c])
                        evt.append(t)
            evv = [t[:, 0 : FGRP * FW].rearrange("p (t r j) -> p t r j",
                                                 t=FGRP, r=4)
                   for t in evt]
            # needed only by the scan/proj phases; keep behind the event DMAs
            ident = cst.tile([128, 128], f32)
            nc.sync.dma_start(ident[:], id_d[:])
            pats = cst.tile([128, 2 * BLKF + 2 * TP], f32)
            nc.sync.dma_start(pats[:], pat_d[:])
            p1 = pats[:, 0:BLKF]                     # d1 pattern, 0 at seg starts
            p2 = pats[:, BLKF:2 * BLKF]              # d2 pattern (d2 == ds)
            pch = pats[:, 2 * BLKF : 2 * BLKF + 2 * TP]  # ds, 80-col segments

            # hidden-weight ring: per CHUNK k the (tile, col-offset) holding
            # [hi | lo] 1024 cols. Chunks 0..2*NWP-1 live pairwise in the wp
            # ring; the rest stream as half tiles through the event slots
            # (same 2 KB slot size), self-gated by the conv dj=2 releases.
            NWP = NCH // 2 - 16
            wts = [wp.tile([96, 4 * HID], f16, tag="wh", name=f"wh{kk}")
                   for kk in range(NWP)]
            whalf = [evp.tile([96, 1024], f16, tag="evs", bufs=16, name=f"whf{i}")
                     for i in range(32)]
            wchunk = []
            for k in range(NCH):
                kk, s = divmod(k, 2)
                if kk < NWP:
                    wchunk.append((wts[kk], 2 * s * HID))
                else:
                    wchunk.append((whalf[2 * (kk - NWP) + s], 0))

            # ---------------- conv: EC -> psum -> a1 (scaled) -------------
            # evac applies (d1-d2)/CONV_SC so the scan cascade yields enc directly
            a1 = accp.tile([96, NCH * BT], f32)
            # (b, k, t) layout: contiguous [96, CHB*T] runs per (block, b)
            a1v = a1[:].rearrange("p (b k t) -> p b k t", b=BL, k=NCH)
            with nc.named_scope("conv"):
                for dj in range(3):
                    for b in range(BL):
                        for g in range(NFG):
                            ps = cps.tile([96, FGRP * 32], f32)
                            t0 = g * FGRP
                            first = True
                            for c in range(2):
                                for q in range(4):
                                    r0 = (q + dj - 1) % 4
                                    j0 = (q + dj - 1) // 4
                                    rhs = evv[(b * NFG + g) * 2 + c][
                                        :, :, r0, j0 + 1 : j0 + 33]
                                    for wt in (ahi, alo):
                                        nc.tensor.matmul(
                                            ps[:],
                                            wt[:, (c * 4 + q) * 96 : (c * 4 + q + 1) * 96],
                                            rhs,
                                            start=first,
                                            stop=(c == 1 and q == 3 and wt is alo),
                                        )
                                        first = False
                            dst = a1v[:, b, dj * 32 : dj * 32 + 32, t0 : t0 + FGRP]
                            srcp = ps[:].rearrange("p (t j) -> p j t", t=FGRP)
                            nc.scalar.activation(dst, srcp, F_.Copy,
                                                 scale=(d1 - d2) / CONV_SC)

            # ---------------- cascade scans: z = scan_d2(scan_d1(EC)) -----
            # per (feature-block, batch) so each starts as soon as its conv
            # evacs land; halves the post-conv scan tail
            HBF = BLKF // 2
            with nc.named_scope("scans"):
                for fb in range(FB):
                    for b in range(BL):
                        c0 = (b * NCH + fb * CHB) * T
                        blk2 = slice(c0, c0 + HBF)
                        ytmp = accp.tile([96, HBF], f32, tag="ytmp", bufs=1,
                                         name="ytmp")
                        nc.vector.tensor_tensor_scan(
                            ytmp[:], p1[0:96, 0:HBF], a1[:, blk2], 0.0,
                            op0=A_.mult, op1=A_.add)
                        nc.vector.tensor_tensor_scan(
                            a1[:, blk2], p2[0:96, 0:HBF], ytmp[:], 0.0,
                            op0=A_.mult, op1=A_.add)
                    if fb == 1:
                        # release the hidden-weight DMA stream: tiny GpSimd
                        # touches read the just-finished z block, the DMAs
                        # overwrite (WAW) - so no weight traffic before here.
                        # The event-slot tiles (kk >= NWP) are gated by their
                        # slot release (conv dj=2) instead.
                        gcol = (NCH + 2 * CHB) * T - 1
                        gate = a1[0:1, gcol : gcol + 1]
                        for kk in range(NWP):
                            nc.gpsimd.tensor_copy(wts[kk][0:1, 0:1], gate)
                            nc.sync.dma_start(wts[kk][:], whl_d[kk])
                        for kk in range(NWP, NCH // 2):
                            for s in range(2):
                                nc.sync.dma_start(
                                    whalf[2 * (kk - NWP) + s][:],
                                    whl_d[kk, :, 1024 * s : 1024 * (s + 1)])
            # now a1[:, k, b, t] = z[t]; enc[t] = z[t-1], enc[0] = 0
            if debug_taps:
                nc.sync.dma_start(taps["enc"][:], a1[:])

            # ---------------- m_in LIF scan -> sign spikes ----------------
            # m_t = dm*m_{t-1}*(m_{t-1}<TH) + enc[t];  s_t = Sign(m_t - TH)
            # t=0: all zero -> sign -1 (memset). t=1: m_1 = enc[1] = z[0].
            Ssg = accp.tile([96, NCH * BT], f16)
            m_in = [stp.tile([96, NCH * BL], f32, tag=f"m_in{i}", name=f"m_in{i}") for i in range(3)]
            x_in = [stp.tile([96, NCH * BL], f32, tag=f"x_in{i}", name=f"x_in{i}") for i in range(2)]
            # all (b, k) ordered to match the a1 (b,k,t) layout
            sgv = Ssg[:].rearrange("p (k b t) -> p b k t", k=NCH, b=BL)
            mv = [m[:].rearrange("p (b k) -> p b k", b=BL) for m in m_in]
            xv = [x[:].rearrange("p (b k) -> p b k", b=BL) for x in x_in]
            with nc.named_scope("m_in"):
                nc.vector.memset(sgv[:, :, :, 0], -1.0)
                nc.scalar.sign(sgv[:, :, :, 1], a1v[:, :, :, 0], bias=negTH[0:96])
                for t in range(2, T):
                    prev = a1v[:, :, :, 0] if t == 2 else mv[(t - 1) % 3]
                    nc.vector.scalar_tensor_tensor(      # x = (m<TH)*m
                        x_in[t % 2][:], prev, TH, prev,
                        op0=A_.is_lt, op1=A_.mult)
                    nc.vector.scalar_tensor_tensor(      # m' = x*dm + enc[t]
                        mv[t % 3], xv[t % 2], dm, a1v[:, :, :, t - 1],
                        op0=A_.mult, op1=A_.add)
                    nc.scalar.sign(sgv[:, :, :, t], mv[t % 3], bias=negTH[0:96])
                    if t % 3 == 0:
                        # tiny dependent matmul keeps the PE HAM clock warm
                        # through this PE-idle phase (else proj runs at 1.2GHz)
                        wrm = tps.tile([16, 8], f32, tag="po", bufs=1)
                        nc.tensor.matmul(wrm[:], m_in[t % 3][0:96, 0:16],
                                         m_in[t % 3][0:96, 0:8],
                                         start=True, stop=True)
            if debug_taps:
                nc.sync.dma_start(taps["sg"][:], Ssg[:])

            # ---------------- hidden projection matmul -------------------
            with nc.named_scope("proj"):
                phi = mps.tile([BT, HID], f32, tag="phi")
                plo = mps.tile([BT, HID], f32, tag="plo")
                korder = (list(range(0, 2 * NWP)) + list(range(64, NCH))
                          + list(range(2 * NWP, 64)))
                for i, k in enumerate(korder):
                    wt, off = wchunk[k]
                    lhs = Ssg[:, k * BT : (k + 1) * BT]
                    nc.tensor.matmul(phi[:], lhs, wt[:, off : off + HID],
                                     start=(i == 0), stop=(i == NCH - 1))
                    nc.tensor.matmul(plo[:], lhs, wt[:, off + HID : off + 2 * HID],
                                     start=(i == 0), stop=(i == NCH - 1))
                projsb = stp.tile([BT, HID], f32)
                nc.scalar.copy(projsb[:], phi[:])
                nc.vector.scalar_tensor_tensor(          # proj += plo * 2^-11
                    projsb[:], plo[:], 1.0 / LO_SC, projsb[:], op0=A_.mult, op1=A_.add)
                if debug_taps:
                    nc.sync.dma_start(taps["pj"][:], projsb[:])

                # transpose proj -> padded (k,b,t80) layout; csh bias at evac
                projTP = stp.tile([128, 4 * BL * TP], f32)
                nc.vector.memset(projTP[:], 0.0)
                pjv = projTP[:].rearrange("p (k b t) -> p k b t", k=4, b=BL)
                id120 = ident[0:BT, 0:BT]
                for k in range(4):
                    pst = tps.tile([128, BT], f32, tag="tp")
                    nc.tensor.transpose(pst[:], projsb[:, k * 128 : (k + 1) * 128], id120)
                    nc.scalar.activation(
                        pjv[:, k, :, 0:T],
                        pst[:].rearrange("p (b t) -> p b t", b=BL),
                        F_.Identity, bias=csh[:, k : k + 1], scale=1.0)
                # chP: k=0..3 -> c_h scan of proj; k=4 -> c_o (written at +LAG)
                chP = stp.tile([128, 5 * BL * TP], f32)
                chv = chP[:].rearrange("p (k b t) -> p k b t", k=5, b=BL)
                for k in range(4):
                    nc.vector.tensor_tensor_scan(
                        chP[:, k * BL * TP : (k + 1) * BL * TP], pch,
                        projTP[:, k * BL * TP : (k + 1) * BL * TP],
                        0.0, op0=A_.mult, op1=A_.add)
                nc.vector.memset(chP[:, 4 * BL * TP : 5 * BL * TP], 0.0)

            # ------- fused m_h + m_o LIF scan (m_o lagged by LAG steps) ----
            # state [128, (k5, b)]: k=0..3 hidden (all 128 partitions),
            # k=4 output (partitions 0..10). Out-proj runs in 15-step blocks
            # as the chain crosses each block boundary, writing c_o at +LAG.
            Ssho = stp.tile([128, 5 * BL * TP], f16)
            shv = Ssho[:].rearrange("p (k b t) -> p k b t", k=5, b=BL)
            m_h = [stp.tile([128, 5 * BL], f32, tag=f"m_h{i}", name=f"m_h{i}") for i in range(3)]
            x_h = stp.tile([128, 5 * BL], f32)
            mhv = [m[:].rearrange("p (k b) -> p k b", k=5) for m in m_h]
            xhv = x_h[:].rearrange("p (k b) -> p k b", k=5)
            with nc.named_scope("m_h"):
                nc.vector.memset(shv[:, :, :, 0], -1.0)
                nc.scalar.sign(shv[:, :, :, 1], chv[:, :, :, 1], bias=negTH[:])
                for t in range(2, T + LAG):
                    prev = chv[:, :, :, 1] if t == 2 else mhv[(t - 1) % 3]
                    nc.vector.scalar_tensor_tensor(
                        x_h[:], prev, TH, prev, op0=A_.is_lt, op1=A_.mult)
                    nc.vector.scalar_tensor_tensor(
                        mhv[t % 3], xhv, dm, chv[:, :, :, t], op0=A_.mult, op1=A_.add)
                    nc.scalar.sign(shv[:, :, :, t], mhv[t % 3], bias=negTH[:])
                    if t in (14, 29, 44, 59):
                        g = t // 15
                        tb = slice(15 * g, 15 * g + 15)
                        pho = tps.tile([OUT, BL * 15], f32, tag="po", bufs=1)
                        plo2 = tps.tile([OUT, BL * 15], f32, tag="po2", bufs=1)
                        for k in range(4):
                            mov = shv[:, k, :, tb]
                            nc.tensor.matmul(pho[:].rearrange("p (b t) -> p b t", b=BL),
                                             wohi[:, k * OUT : (k + 1) * OUT], mov,
                                             start=(k == 0), stop=(k == 3))
                            nc.tensor.matmul(plo2[:].rearrange("p (b t) -> p b t", b=BL),
                                             wolo[:, k * OUT : (k + 1) * OUT], mov,
                                             start=(k == 0), stop=(k == 3))
                        codst = chv[0:OUT, 4, :, LAG + 15 * g : LAG + 15 * g + 15]
                        nc.scalar.activation(
                            codst, pho[:].rearrange("p (b t) -> p b t", b=BL),
                            F_.Identity, bias=cso[:], scale=1.0)
                        nc.vector.scalar_tensor_tensor(
                            codst, plo2[:].rearrange("p (b t) -> p b t", b=BL),
                            1.0 / LO_SC, codst, op0=A_.mult, op1=A_.add)
                        for b in range(BL):
                            cob = chv[0:OUT, 4, b, LAG + 15 * g : LAG + 15 * g + 15]
                            init = (0.0 if g == 0 else
                                    chv[0:OUT, 4, b, LAG + 15 * g - 1 : LAG + 15 * g])
                            nc.vector.tensor_tensor_scan(
                                cob, p2[0:OUT, 1:16], cob, init,
                                op0=A_.mult, op1=A_.add)

            # ---------------- spike count + output -----------------------
            with nc.named_scope("out"):
                accT = stp.tile([OUT, BL], f32)
                nc.vector.tensor_reduce(
                    accT[:], shv[0:OUT, 4, :, LAG : LAG + T],
                    axis=mybir.AxisListType.X, op=A_.add)
                outsb = stp.tile([OUT, BL], f32)
                nc.scalar.activation(outsb[:], accT[:], F_.Copy,
                                     bias=0.5, scale=1.0 / (2.0 * T))
                nc.sync.dma_start(out_d[:].rearrange("b o -> o b"), outsb[:])

    nc.finalize()
    return nc


def _host_prep(events, w_conv, w_hid, w_out):
    import ml_dtypes
    ev = np.clip(events, 0, 1).astype(np.float16)          # [B,T,2,H,W] {0,1}
    # residue-shuffled, h-major, margin-padded: evr[h,b,t,c,r,1+j] = ev[b,t,c,h,4j+r]
    evr = np.zeros((128, B, T, 2, 4, 34), np.float16)
    evr[:, :, :, :, :, 1:33] = (
        ev.reshape(B, T, 2, H, 32, 4).transpose(3, 0, 1, 2, 5, 4))
    # -> per-(b,g,c) contiguous tiles: [B, NFG, 2, 128, FGRP*4*34]
    evr = (evr.reshape(128, B, NFG, FGRP, 2, 4 * 34)
              .transpose(1, 2, 4, 0, 3, 5)
              .reshape(B, NFG * 2, 128, FGRP * 4 * 34))
    evr = evr.astype(ml_dtypes.float8_e4m3fn)

    wc = (np.asarray(w_conv, np.float64) * SCALE * CONV_SC).astype(np.float32)
    A = np.zeros((8, 128, 96), np.float32)
    for c in range(2):
        for q in range(4):
            for di in range(3):
                for i in range(32):
                    for p in range(4):
                        h = 4 * i + di - 1 + p
                        if 0 <= h < 128:
                            A[c * 4 + q, h, di * 32 + i] = wc[c, p, q]
    ahi = A.astype(np.float16)
    alo = (A - ahi.astype(np.float32)).astype(np.float16)

    Wh = (np.asarray(w_hid, np.float64) / 2.0).astype(np.float32)   # [512, 9216]
    WT = np.ascontiguousarray(Wh.T)                                  # [9216, 512]
    djs, js = np.divmod(np.arange(NCH), 32)
    dis, is_ = np.divmod(np.arange(96), 32)
    fmap = ((dis[None, :] * 3 + djs[:, None]) * 1024
            + is_[None, :] * 32 + js[:, None])                       # [NCH, 96]
    whP = WT[fmap]                                                   # [NCH, 96, 512]
    whi = whP.astype(np.float16)
    wlo = ((whP - whi.astype(np.float32)) * LO_SC).astype(np.float16)
    whl = np.empty((NCH // 2, 96, 4 * HID), np.float16)
    for s in range(2):
        whl[:, :, 2 * s * HID : (2 * s + 1) * HID] = whi[2 * np.arange(NCH // 2) + s]
        whl[:, :, (2 * s + 1) * HID : (2 * s + 2) * HID] = wlo[2 * np.arange(NCH // 2) + s]
    csh = Wh.sum(axis=1).astype(np.float32).reshape(4, 128)

    WoC = (np.asarray(w_out, np.float64).T / 2.0).astype(np.float32).reshape(4, 128, OUT)
    wohi = WoC.astype(np.float16)
    wolo = ((WoC - wohi.astype(np.float32)) * LO_SC).astype(np.float16)
    cso = (np.asarray(w_out, np.float64).sum(axis=1) / 2.0).astype(np.float32)

    # scan decay patterns: [0, d, d, ..., d] per T-length segment
    BLKF = CHB * BL * T
    TP, LAG = 80, 18
    pats = np.zeros((128, 2 * BLKF + 2 * TP), np.float32)
    seg1 = np.full(T, d1, np.float32); seg1[0] = 0.0
    seg2 = np.full(T, d2, np.float32); seg2[0] = 0.0
    pats[:, 0:BLKF] = np.tile(seg1, CHB * BL)[None, :]
    pats[:, BLKF:2 * BLKF] = np.tile(seg2, CHB * BL)[None, :]
    segp = np.zeros(TP, np.float32); segp[1:T] = d2
    pats[:, 2 * BLKF : 2 * BLKF + 2 * TP] = np.tile(segp, 2)[None, :]

    shared = dict(ahi=ahi, alo=alo, whl=whl, csh=csh, wohi=wohi, wolo=wolo,
                  cso=cso, ident=np.eye(128, dtype=np.float32), pats=pats)
    return evr, shared


def _run(events, w_conv, w_hid, w_out, debug_taps=False, trace=False, ncores=NCORES):
    key = ("nc", debug_taps)
    if key not in _CACHE:
        _CACHE[key] = _build_program(debug_taps=debug_taps)
    nc = _CACHE[key]
    ev, shared = _host_prep(events, w_conv, w_hid, w_out)
    in_maps = []
    for c in range(ncores):
        m = {"ev": np.ascontiguousarray(
            ev[c * BL : (c + 1) * BL].reshape(BL * NFG * 2, 128, FGRP * FW))}
        m.update(shared)
        in_maps.append(m)
    res = run_bass_kernel_spmd(nc, in_maps, core_ids=list(range(ncores)), trace=trace)
    out = np.concatenate([res.results[c]["out"] for c in range(ncores)], axis=0)
    return out.astype(np.float32), res


def kernel(events, w_conv, w_hid, w_out, batch_size):
    out, _ = _run(np.asarray(events), np.asarray(w_conv),
                  np.asarray(w_hid), np.asarray(w_out))
    return out
